# revision 40
# baseline (speedup 1.0000x reference)
"""Multi-head attention + output projection on 8 Trainium2 NeuronCores.

Problem (hardcoded): B=4, N=M=2048, D_IN=D_OUT=512, H=8, HD=VD=64.
  out = softmax(q @ k^T / sqrt(64)) @ v, heads concat, @ W_out.T + b_out

Sharding: each core owns (batch b = core//2, query-row half = core%2):
  q-chunk [1024, 512], full k/v for that batch, full W_out. All 8 heads are
  computed locally, so no collectives are needed; the host concatenates the
  8 disjoint [1024, 512] output chunks.

Device algorithm per core (S^T layout; matmuls stream 1 row/cycle at
free-size>=256 in f32r/bf16, so PE time = total output free elements).
64 rounds, one per (head-pair, j-tile of 128 keys):
    S_T[j,i] = k_h^T q_h                (K=64 QK matmuls; PSUM tiles are
               per-head AND per-i-half because PSUM deps are whole-tile -
               four 1-bank tiles give the sub-tile WAR/RAW granularity the
               software pipeline needs)
    even head: P0 = exp(0.125*S0)       (ScalarE activation, per-512 halves)
    odd head:  P1 = Schraudolph exp     (VectorE tensor_scalar bit-trick:
               int16(S1*(2^7/ln2/8) + (127*2^7 - 7.42)) bitcast to BF16;
               one DVE op per half. ~1.8% rms weight error -> ~1.3e-2
               output rel err, within the 2e-2 gate. The f32 variant of the
               trick is rejected by the BIR verifier (f32r inputs must come
               from a rounding producer) and f32r lhsT cannot mix with bf16
               rhs, so the odd heads get a bf16 copy of [v|1] (vb))
    O_aug[65,i] += [v_h | 1]^T @ P_h    (PSUM o0/o1; row 64 = sumexp)
  Round r emits [exps(r), QK(r+1), PV(r-1)]: the PE consumes P tiles
  finished a full round earlier so it never waits on the exp engines
  (1.704us/round of matmul vs ~1.3us/round on each exp engine), and the
  flat round stream crosses pair boundaries seamlessly. Normalization of
  pair p (PSUM drain copies on ScalarE+VectorE, DVE reciprocal of the
  sumexp row, SP partition-move to row 0, gpsimd partition-broadcast and
  gpsimd multiplies) is woven into rounds 1-5 of pair p+1 so it never
  blocks the exp streams. Pair 3 is normalized at the tail in i-quarters:
  ScalarE copies O out of PSUM while the DVE reciprocal reads the PSUM
  sumexp row directly, a K=1 ones matmul broadcasts the recip row into the
  spare 1-bank PSUM slots, and the DVE multiply feeds the projection,
  which contracts each pair in one K=128 matmul (pair 3 unfused: K=65 with
  the bias riding the ones row, plus stg7 K=64) interleaved per quarter.
  The final chunk splits its PSUM->SBUF copy and its store across two
  engines/DMA queues to shorten the drain.
  Cost-model exec ~131.3 us/core (PE busy ~118.4 us is the floor: QK+PV
  stream 2*M*N/128 cycles/head at 2.4 GHz + 9.4 us projection; the rest is
  DMA-ceremony startup ~3.4, pipeline fill ~1.7, drain ~4.5); measured HW
  rel err ~1.26e-2 vs the fp32 reference (gate 2e-2).
"""

import numpy as np

B, N, M, D, H, HD = 4, 2048, 2048, 512, 8, 64
NLOC = N // 2  # query rows per core
NCORES = 8
VA_C = 66  # per-head packed v columns: 64 v + 1 ones + 1 pad

# Schraudolph exp bit-trick constants (exp(0.125*s) via int16->bf16
# bitcast; the f32r variant trips the BIR verifier's "rounded to FP32r"
# producer rule, bf16 does not and also streams 1 row/cycle in the PE)
EXP_A = float(2.0**7 / np.log(2.0) * 0.125)
EXP_B = float(127 * 2**7 - 7.42)


def _build_bass(debug=False):
    import concourse.mybir as mybir
    import concourse.tile as tile
    from concourse import bacc

    f32 = mybir.dt.float32
    f32r = mybir.dt.float32r
    i16 = mybir.dt.int16
    bf16 = mybir.dt.bfloat16

    nc = bacc.Bacc()
    qt_d = nc.dram_tensor("qt", [D, NLOC], f32r, kind="ExternalInput")
    kt_d = nc.dram_tensor("kt", [D, M], f32r, kind="ExternalInput")
    va_d = nc.dram_tensor("va", [M, H, VA_C], f32r, kind="ExternalInput")
    vb_d = nc.dram_tensor("vb", [M, H // 2, VA_C], bf16, kind="ExternalInput")
    wt_d = nc.dram_tensor("wt", [128, H // 2, D], f32r, kind="ExternalInput")
    wt7_d = nc.dram_tensor("wt7", [HD, D], f32r, kind="ExternalInput")
    on_d = nc.dram_tensor("on1", [1, NLOC], f32r, kind="ExternalInput")
    wt6b_d = nc.dram_tensor("wt6b", [HD + 1, D], f32r, kind="ExternalInput")
    out_d = nc.dram_tensor("out", [NLOC, D], f32, kind="ExternalOutput")

    JT = M // 128  # 16 j-tiles
    IC = NLOC // 512  # 2 i-chunks for matmul free dim

    with tile.TileContext(nc) as tc:
        with (
            tc.tile_pool(name="persist", bufs=1) as persist,
            tc.tile_pool(name="pt", bufs=3) as ptp,
            tc.tile_pool(name="work", bufs=1) as work,
            tc.tile_pool(name="ps_s", bufs=1, space="PSUM") as ps_s_pool,
            tc.tile_pool(name="ps_o", bufs=1, space="PSUM") as ps_o_pool,
        ):
            # split per o-tile so head-pair 0 can start before all loads land
            qt_sb = [persist.tile([128, NLOC], f32r, tag=f"qt{o}", name=f"qt{o}") for o in range(4)]
            kt_sb = [persist.tile([128, M], f32r, tag=f"kt{o}", name=f"kt{o}") for o in range(4)]
            qt_r = qt_d.rearrange("(o p) i -> o p i", p=128)
            kt_r = kt_d.rearrange("(o p) j -> o p j", p=128)
            va_r = va_d.rearrange("(g t p) h c -> g p t h c", p=128, g=4)
            va_sb = [persist.tile([128, JT // 4, H, VA_C], f32r, tag=f"va{g}", name=f"va{g}") for g in range(4)]
            vb_r = vb_d.rearrange("(g t p) h c -> g p t h c", p=128, g=4)
            vb_sb = [persist.tile([128, JT // 4, H // 2, VA_C], bf16, tag=f"vb{g}", name=f"vb{g}") for g in range(4)]
            # parallel queues for the first-round operands (SP takes both
            # q row-halves back-to-back, the gpsimd SWDGE ring the first k
            # j-tile) so round 0 starts ~1.2us earlier
            nc.sync.dma_start(qt_sb[0][0:HD, 0:512], qt_r[0, 0:HD, 0:512])
            nc.scalar.dma_start(qt_sb[0][HD:128, 0:512], qt_r[0, HD:128, 0:512])
            nc.gpsimd.dma_start(kt_sb[0][:, 0:128], kt_r[0, :, 0:128])
            nc.gpsimd.dma_start(kt_sb[0][:, 128:256], kt_r[0, :, 128:256])
            nc.sync.dma_start(qt_sb[0][:, 512:1024], qt_r[0, :, 512:1024])
            nc.sync.dma_start(kt_sb[0][:, 256:512], kt_r[0, :, 256:512])
            nc.sync.dma_start(va_sb[0][:, 0:1], va_r[0, :, 0:1])
            nc.sync.dma_start(vb_sb[0][:, 0:1], vb_r[0, :, 0:1])
            nc.sync.dma_start(va_sb[0][:, 1:4], va_r[0, :, 1:4])
            nc.sync.dma_start(vb_sb[0][:, 1:4], vb_r[0, :, 1:4])
            nc.sync.dma_start(kt_sb[0][:, 512:1024], kt_r[0, :, 512:1024])
            nc.sync.dma_start(kt_sb[0][:, 1024:2048], kt_r[0, :, 1024:2048])
            # v groups cover j-tiles for ALL pairs: pair 0 streams through all
            # four within its first 33us, so they go before later pairs' q/k
            for g in range(1, 4):
                nc.sync.dma_start(va_sb[g], va_r[g])
                nc.sync.dma_start(vb_sb[g], vb_r[g])
            for o in range(1, 4):
                nc.sync.dma_start(kt_sb[o], kt_r[o])
                nc.sync.dma_start(qt_sb[o], qt_r[o])
            wt_sb = persist.tile([128, H // 2, D], f32r)
            nc.sync.dma_start(wt_sb, wt_d[:])
            wt7_sb = persist.tile([HD, D], f32r)
            nc.sync.dma_start(wt7_sb, wt7_d[:])
            wt6b_sb = persist.tile([HD + 1, D], f32r)
            nc.sync.dma_start(wt6b_sb, wt6b_d[:])

            # normalized O^T packed per head-pair: rows 0-63 even head,
            # rows 64-127 odd head (moved there by a partition-shift DMA) so
            # the projection contracts both heads in one K=128 matmul.
            ot_sb = persist.tile([128, H // 2, NLOC], f32r)
            nc.sync.dma_start(ot_sb[HD : HD + 1, 3, :], on_d[:])
            stg7 = persist.tile([HD, NLOC], f32r)

            def va_lhs(jt, h):
                return va_sb[jt // 4][:, jt % 4, h, 0 : HD + 1]

            def vb_lhs(jt, hp):
                return vb_sb[jt // 4][:, jt % 4, hp, 0 : HD + 1]

            def emit_qk(hp, jt, interleave=False):
                # separate PSUM tiles per i-half: PSUM deps are whole-tile,
                # so per-half tiles are what lets exp(ic0) overlap QK(ic1).
                # Round 0 interleaves head/ic order to match DMA arrival.
                jsl = slice(jt * 128, (jt + 1) * 128)
                s0 = [None, None]
                s1 = [None, None]

                def one(h, ic):
                    tgt = s0 if h == 0 else s1
                    tgt[ic] = ps_s_pool.tile(
                        [128, 512], f32, tag=f"s{h}{ic}", name="s"
                    )
                    rb = slice(0, HD) if h == 0 else slice(HD, 128)
                    nc.tensor.matmul(
                        tgt[ic],
                        lhsT=kt_sb[hp][rb, jsl],
                        rhs=qt_sb[hp][rb, slice(ic * 512, (ic + 1) * 512)],
                        start=True,
                        stop=True,
                    )

                order = [(0, 0), (1, 0), (0, 1), (1, 1)] if interleave else [
                    (0, 0), (0, 1), (1, 0), (1, 1)]
                for h, ic in order:
                    one(h, ic)
                return s0, s1

            def emit_exps(s0, s1):
                # even head: true exp on ScalarE; odd head: Schraudolph
                # bit-trick on VectorE. Both emitted per-512 halves so the
                # next QK's sub-tile writes pipeline against them.
                p0 = ptp.tile([128, NLOC], f32r, tag="p0")
                for ic in range(IC):
                    isl = slice(ic * 512, (ic + 1) * 512)
                    nc.scalar.activation(
                        p0[:, isl], s0[ic],
                        mybir.ActivationFunctionType.Exp, scale=0.125,
                    )
                p1 = ptp.tile([128, NLOC], i16, tag="p1")
                for ic in range(IC):
                    isl = slice(ic * 512, (ic + 1) * 512)
                    nc.vector.tensor_scalar(
                        p1[:, isl], s1[ic], EXP_A, EXP_B,
                        mybir.AluOpType.mult, mybir.AluOpType.add,
                    )
                return p0, p1

            def emit_pv(hp, jt, p0, p1, ps_o0, ps_o1):
                for ic in range(IC):
                    isl = slice(ic * 512, (ic + 1) * 512)
                    nc.tensor.matmul(
                        ps_o0[:, isl],
                        lhsT=va_lhs(jt, 2 * hp),
                        rhs=p0[:, isl],
                        start=(jt == 0),
                        stop=(jt == JT - 1),
                    )
                for ic in range(IC):
                    isl = slice(ic * 512, (ic + 1) * 512)
                    nc.tensor.matmul(
                        ps_o1[:, isl],
                        lhsT=vb_lhs(jt, hp),
                        rhs=p1[:, isl].bitcast(bf16),
                        start=(jt == 0),
                        stop=(jt == JT - 1),
                    )

            # deferred normalization of pair hp, woven into pair hp+1's
            # rounds so its DVE/Pool work never blocks the exp stream.
            # stage 0a (DVE, before exps): copy odd-head O out of PSUM
            # stage 0b (ACT, after exps):  copy even-head O out of PSUM
            # stage 1  (DVE): reciprocals of the sumexp rows; SP moves the
            #                 recip row to partition 0 (gpsimd's ucode reads
            #                 the tile's partition 0, ignoring the AP base)
            # stage 2  (Pool): partition-broadcast to 64 rows
            # stage 3  (Pool): O * recip -> ot_sb / staging
            # stage 4  (SP):  odd-head staging -> ot_sb rows 64:127
            def make_norm(hp, ps_o0, ps_o1):
                st = {}

                def pre0():
                    st["oc0"] = work.tile([HD + 1, NLOC], f32, tag="oc0", name="oc0")
                    nc.scalar.copy(st["oc0"], ps_o0)
                    st["oc1"] = work.tile([HD + 1, NLOC], f32, tag="oc1", name="oc1")
                    nc.vector.tensor_copy(st["oc1"], ps_o1)

                def post2():
                    for p, oc in (("1", st["oc1"]), ("0", st["oc0"])):
                        rc = work.tile([HD + 1, NLOC], f32, tag=f"rc{p}", name="rc")
                        nc.vector.reciprocal(rc[HD : HD + 1, :], oc[HD : HD + 1, :])
                        rcm = work.tile([1, NLOC], f32, tag=f"rcm{p}", name="rcm")
                        nc.sync.dma_start(rcm, rc[HD : HD + 1, :])
                        st[f"rcm{p}"] = rcm

                def post3():
                    for p in ("1", "0"):
                        bc = work.tile([HD, NLOC], f32, tag=f"bc{p}", name="bc")
                        nc.gpsimd.partition_broadcast(bc, st[f"rcm{p}"])
                        st[f"bc{p}"] = bc

                def post4():
                    stg = work.tile([HD, NLOC], f32r, tag="stg", name="stg")
                    nc.gpsimd.tensor_tensor(
                        stg, st["oc1"][0:HD, :], st["bc1"], mybir.AluOpType.mult
                    )
                    st["stg"] = stg
                    nc.gpsimd.tensor_tensor(
                        ot_sb[0:HD, hp, :], st["oc0"][0:HD, :], st["bc0"],
                        mybir.AluOpType.mult,
                    )

                def post5():
                    nc.sync.dma_start(ot_sb[HD:128, hp, :], st["stg"])

                return {"pre0": pre0, "post2": post2, "post3": post3,
                        "post4": post4, "post5": post5}

            # flat 64-round schedule: round r emits [exps(r), QK(r+1),
            # PV(r-1)] so the PE consumes P tiles finished a full round
            # earlier and never waits on the exp engines; sub-tile WAR lets
            # the exps of round r drain while round r+1's QK halves land.
            SQ = [(hp, jt) for hp in range(4) for jt in range(JT)]
            pair_ps = {}

            def get_ps(hp):
                if hp not in pair_ps:
                    o0 = ps_o_pool.tile([HD + 1, NLOC], f32, tag="o0", name="o0")
                    o1 = ps_o_pool.tile([HD + 1, NLOC], f32, tag="o1", name="o1")
                    pair_ps[hp] = (o0, o1)
                return pair_ps[hp]

            def emit_rounds():
                norms = {}
                s_cur = emit_qk(*SQ[0], interleave=True)
                prev_p = None
                for r in range(len(SQ)):
                    hp, jt = SQ[r]
                    if jt == 0 and hp > 0:
                        norms[hp - 1] = make_norm(hp - 1, *get_ps(hp - 1))
                    nv = norms.get(hp - 1) if jt <= 5 else None
                    if nv and jt == 1:
                        nv["pre0"]()   # ACT copy0 + DVE copy1 ahead of exps
                    p_cur = emit_exps(*s_cur)
                    if nv and 2 <= jt <= 5:
                        nv[f"post{jt}"]()
                    if r + 1 < len(SQ):
                        s_nxt = emit_qk(*SQ[r + 1])
                    if r >= 1:
                        php, pjt = SQ[r - 1]
                        emit_pv(php, pjt, *prev_p, *get_ps(php))
                    prev_p = p_cur
                    s_cur = s_nxt
                php, pjt = SQ[-1]
                emit_pv(php, pjt, *prev_p, *get_ps(php))

            def proj_mms(ps_f, csl, esl):
                for hp in range(3):
                    nc.tensor.matmul(
                        ps_f,
                        lhsT=(ot_sb[:, hp, csl]),
                        rhs=(wt_sb[:, hp, esl]),
                        start=(hp == 0),
                        stop=False,
                    )
                nc.tensor.matmul(
                    ps_f, lhsT=(ot_sb[0 : HD + 1, 3, csl]), rhs=(wt6b_sb[:, esl]),
                    start=False, stop=False,
                )
                nc.tensor.matmul(
                    ps_f, lhsT=(stg7[:, csl]), rhs=(wt7_sb[:, esl]),
                    start=False, stop=True,
                )

            def proj_chunks(icc_range):
                for icc in icc_range:
                    csl = slice(icc * 128, (icc + 1) * 128)
                    ps_f = ps_s_pool.tile([128, D], f32, tag=f"s{icc % 2}0", name="ps_f")
                    proj_mms(ps_f, csl, slice(0, D))
                    f_sb = ptp.tile([128, D], f32, tag="fin", bufs=8, name="f_sb")
                    if icc == 7:
                        # last chunk: split copy across both engines and the
                        # store across two DMA queues - only ~0.4us of copy
                        # and half a transfer sit on the critical tail
                        nc.vector.tensor_copy(f_sb[:, 0:256], ps_f[:, 0:256])
                        nc.scalar.copy(f_sb[:, 256:512], ps_f[:, 256:512])
                        nc.sync.dma_start(out_d[csl, 0:256], f_sb[:, 0:256])
                        nc.scalar.dma_start(out_d[csl, 256:512], f_sb[:, 256:512])
                    elif icc == 6:
                        # on DVE so the ScalarE is free for chunk 7's half
                        # copy the moment its matmuls finish
                        nc.vector.tensor_copy(f_sb, ps_f)
                        nc.sync.dma_start(out_d[csl, :], f_sb)
                    else:
                        # ScalarE only: the DVE queue carries the norm chain
                        # (recips + mults) the proj critically waits on
                        nc.scalar.copy(f_sb, ps_f)
                        nc.sync.dma_start(out_d[csl, :], f_sb)

            # tail: normalize pair 3 straight from PSUM in i-quarters,
            # interleaved with the projection. The recip row is broadcast to
            # 64 partitions by a K=1 PE matmul (ones row from ot_sb x recip
    
            # row) into the spare 1-bank s01/s11 PSUM slots - no partition-
            # move DMA or gpsimd on the critical tail.
            def tail_norm_q(q, ps_o0, ps_o1):
                qsl = slice(q * 256, (q + 1) * 256)
                mults = []
                for p, ps in (("1", ps_o1), ("0", ps_o0)):
                    # O quarter to SBUF on the idle ScalarE, in parallel with
                    # the DVE recip straight off the PSUM sumexp row
                    oc = work.tile([HD + 1, 256], f32, tag=f"oc{p}q", name="ocq")
                    nc.scalar.copy(oc, ps[:, qsl])
                    rc = work.tile([HD + 1, NLOC], f32r, tag=f"rc{p}", name="rc")
                    with nc.allow_low_precision(reason="recip row rounded to f32r for the bcast matmul"):
                        nc.vector.reciprocal(rc[HD : HD + 1, 0:256], ps[HD : HD + 1, qsl])
                    # K=1 ones matmul broadcasts the recip row to 64 rows in
                    # the spare 1-bank s01/s11 PSUM slots (no DMA/gpsimd hop)
                    bc = ps_s_pool.tile([HD, 256], f32, tag=f"s{p}1", name="bcq")
                    nc.tensor.matmul(
                        bc,
                        lhsT=ot_sb[HD : HD + 1, 3, 0:HD],
                        rhs=rc[HD : HD + 1, 0:256],
                        start=True,
                        stop=True,
                    )
                    mults.append((p, oc, bc))
                for p, oc, bc in mults:
                    dst = stg7[:, qsl] if p == "1" else ot_sb[0:HD, 3, qsl]
                    nc.vector.tensor_tensor(
                        dst, oc[0:HD, :], bc, mybir.AluOpType.mult
                    )

            emit_rounds()
            po0, po1 = pair_ps[3]
            tail_norm_q(0, po0, po1)
            for q in range(4):
                if q < 3:
                    tail_norm_q(q + 1, po0, po1)
                proj_chunks(range(q * 2, (q + 1) * 2))

    nc.finalize()
    return nc


def _host_prep(q, k, v, W_out, b_out):
    """Shard + lay out inputs per core (pure layout: transpose/pack)."""
    q = np.asarray(q, dtype=np.float32)
    k = np.asarray(k, dtype=np.float32)
    v = np.asarray(v, dtype=np.float32)
    W_out = np.asarray(W_out, dtype=np.float32)
    b_out = np.asarray(b_out, dtype=np.float32)

    qT = np.ascontiguousarray(q.transpose(0, 2, 1))  # [B, D, N]
    kT = np.ascontiguousarray(k.transpose(0, 2, 1))  # [B, D, M]

    va = np.zeros((B, M, H, VA_C), dtype=np.float32)
    va[..., :HD] = v.reshape(B, M, H, HD)
    va[..., HD] = 1.0
    import ml_dtypes
    vb = va[:, :, 1::2, :].astype(ml_dtypes.bfloat16)  # odd heads for bf16 PV

    # wt[j2, hp, e] = W_out[e, hp*128 + j2] (two heads per 128-row block)
    wt = np.ascontiguousarray(W_out.T.reshape(H // 2, 128, D).transpose(1, 0, 2))

    in_maps = []
    for c in range(NCORES):
        b_, ih = divmod(c, 2)
        in_maps.append(
            {
                "qt": np.ascontiguousarray(qT[b_, :, ih * NLOC : (ih + 1) * NLOC]),
                "kt": kT[b_],
                "va": va[b_],
                "vb": vb[b_],
                "wt": wt,
                "wt7": np.ascontiguousarray(W_out.T[448:512, :]),
                "on1": np.ones((1, NLOC), np.float32),
                "wt6b": np.ascontiguousarray(
                    np.concatenate([W_out.T[384:448, :], b_out[None, :]], axis=0)
                ),
            }
        )
    return in_maps


def kernel(q, k, v, W_out, b_out):
    from concourse.bass_utils import run_bass_kernel_spmd

    nc = _build_bass()
    in_maps = _host_prep(q, k, v, W_out, b_out)
    res = run_bass_kernel_spmd(nc, in_maps, core_ids=list(range(NCORES)))
    out = np.empty((B, N, D), dtype=np.float32)
    for c, r_ in enumerate(res.results):
        b_, ih = divmod(c, 2)
        out[b_, ih * NLOC : (ih + 1) * NLOC, :] = r_["out"]
    return out
